# Initial kernel scaffold
#
"""Trainium2 Bass kernel for nn_MultiHeadAttention (B=2, S=2048, E=1024, H=8, D=128).

Sharding (8 cores): core c handles batch b=c//4 and head-pair g=c%4
(heads 2g, 2g+1 -> E-columns [256g, 256g+256)).
 - Q/K/V projections column-parallel (each core computes its 256 columns).
 - Attention device-local per head, computed in transposed score layout
   scoresT[k, q] so softmaxed weights are directly the rhs of attn@V.
 - Out-projection row-parallel: each core produces a full-shape partial
   out_partial = attn_out_heads @ Wo[rows] in bf16; host sums 4 partials
   per batch and adds the bo / bv-induced bias row (softmax weights sum
   to 1, so the V bias contributes exactly + bv @ Wo per output row).
 - Causal structure: fully-masked (strictly upper) 128x512 blocks are
   skipped; diagonal-straddling 128-wide sub-blocks get the (identical)
   triangular additive mask via a narrow identity matmul; scores/exp/
   attn@V are restricted to the valid column range.
 - PSUM pools are persistent and split (proj 2 / scores 2 / attn-out 2 /
   shared colsum+broadcast+outproj 2 banks) so the tile scheduler can
   overlap iteration i+1's projections with iteration i's attention;
   projection chains use one bank each (drains double-buffer) and are
   ordered to match the attention q-chunk order.
 - Softmax row-sums: non-diagonal k-tile groups pairwise-added to bf16
   partials (Pool first pair, DVE rest); the diagonal group is summed
   piecewise on DVE (valid columns only) and folded into the accumulator,
   leaving a single ones-matmul on the PE for the partition reduce.
"""

import os
import sys

for _p in ("/opt/trn_rl_repo", os.environ.get("TRN_RL_REPO", "")):
    if _p and os.path.isdir(_p) and _p not in sys.path:
        sys.path.insert(0, _p)

import numpy as np
import ml_dtypes

BF16 = ml_dtypes.bfloat16

B, S, E, H = 2, 2048, 1024, 8
D = E // H          # 128
HP = 2              # heads per core
C = HP * D          # 256 projection columns per core
NCORES = 8
KT = S // 128       # 16 k-tiles
QC = S // 512       # 4 q-chunks
SCALE = 1.0 / float(np.sqrt(D))
MASK_NEG = -30000.0

_prog_cache = {}


def build_program(n_iters: int = 1, **opt):
    """Build the SPMD Bass program (Tile). Returns the compiled Bacc object."""
    import concourse.bass as bass
    import concourse.mybir as mybir
    import concourse.tile as tile
    from concourse import bacc, bass_isa
    from concourse.masks import make_identity
    from contextlib import ExitStack

    f32 = mybir.dt.float32
    bf16 = mybir.dt.bfloat16
    AF = mybir.ActivationFunctionType

    o = dict(xt_bufs=16, expt_bufs=2, part_bufs=5, acc_bufs=3, outst_bufs=4,
             proj_bufs=2, sc_bufs=2, ot_bufs=2, csop_bufs=2,
             slim_diag=True, pair_reduce=False, rev_j=False, osb_split=False,
             fuse_rs=False, v_drain="act", tri_sel=False,
             part_pool_eng=True, dve_diag_sum=True,
             partials="pairwise", qk_drain="dve", osb0_pool=False, op_reorder=False, qk_chain="single", hj_alt=False, out_sp=False)
    o.update(opt)

    nc = bacc.Bacc("TRN2", target_bir_lowering=False, debug=False,
                   enable_partition_id=False)

    # ---- DRAM I/O (per-core slices supplied by the host) ----
    xq_t = nc.dram_tensor("xq_t", [E, S], bf16, kind="ExternalInput")
    xk_t = nc.dram_tensor("xk_t", [E, S], bf16, kind="ExternalInput")
    xv_t = nc.dram_tensor("xv_t", [E, S], bf16, kind="ExternalInput")
    wq_d = nc.dram_tensor("wq", [E, C], bf16, kind="ExternalInput")
    wk_d = nc.dram_tensor("wk", [E, C], bf16, kind="ExternalInput")
    wv_d = nc.dram_tensor("wv", [E, C], bf16, kind="ExternalInput")
    wo_d = nc.dram_tensor("wo", [C, E], bf16, kind="ExternalInput")
    bqk_d = nc.dram_tensor("bqk", [128, 4], f32, kind="ExternalInput")
    tri_d = nc.dram_tensor("tri", [128, 128], bf16, kind="ExternalInput")
    out_d = nc.dram_tensor("out", [S, E], bf16, kind="ExternalOutput")

    with tile.TileContext(nc) as tc, ExitStack() as ctx:
        persist = ctx.enter_context(tc.tile_pool(name="persist", bufs=1))
        xt_pool = ctx.enter_context(tc.tile_pool(name="xt", bufs=o["xt_bufs"]))
        expt_pool = ctx.enter_context(tc.tile_pool(name="expt",
                                                   bufs=o["expt_bufs"]))
        acc_pool = ctx.enter_context(tc.tile_pool(name="acc",
                                                  bufs=o["acc_bufs"]))
        part_pool = ctx.enter_context(tc.tile_pool(name="part",
                                                   bufs=o["part_bufs"]))
        outst = ctx.enter_context(tc.tile_pool(name="outst",
                                               bufs=o["outst_bufs"]))
        # persistent PSUM pools: 2+2+2+2 = 8 banks
        ps_proj = ctx.enter_context(
            tc.tile_pool(name="ps_proj", bufs=o["proj_bufs"], space="PSUM"))
        ps_sc = ctx.enter_context(
            tc.tile_pool(name="ps_sc", bufs=o["sc_bufs"], space="PSUM"))
        ps_ot = ctx.enter_context(
            tc.tile_pool(name="ps_ot", bufs=o["ot_bufs"], space="PSUM"))
        ps_csop = ctx.enter_context(
            tc.tile_pool(name="ps_csop", bufs=o["csop_bufs"], space="PSUM"))

        # ---- constants ----
        ident = persist.tile([128, 128], bf16, tag="ident")
        make_identity(nc, ident)
        ones_col = persist.tile([128, 1], f32, tag="ones_col")
        nc.vector.memset(ones_col, 1.0)
        ones_row = persist.tile([1, 128], f32, tag="ones_row")
        nc.vector.memset(ones_row, 1.0)
        ones_col_bf = persist.tile([128, 1], bf16, tag="ones_col_bf")
        nc.vector.memset(ones_col_bf, 1.0)
        ones_row_bf = persist.tile([1, 128], bf16, tag="ones_row_bf")
        nc.vector.memset(ones_row_bf, 1.0)

        # ---- persistent weight / bias / mask tiles ----
        wq_sb = persist.tile([128, 8, C], bf16, tag="wq")
        wk_sb = persist.tile([128, 8, C], bf16, tag="wk")
        wv_sb = persist.tile([128, 8, C], bf16, tag="wv")
        wo_sb = persist.tile([128, HP, E], bf16, tag="wo")
        bqk = persist.tile([128, 4], f32, tag="bqk")
        tri_sb = persist.tile([128, 128], bf16, tag="tri")

        # late-needed loads on the SWDGE queue so they don't block the
        # activation stream on the HWDGE queue
        nc.gpsimd.dma_start(out=tri_sb, in_=tri_d.ap())
        nc.gpsimd.dma_start(out=wo_sb,
                            in_=wo_d.ap().rearrange("(h p) n -> p h n", p=128))
        nc.sync.dma_start(out=bqk, in_=bqk_d.ap())
        nc.sync.dma_start(
            out=wq_sb, in_=wq_d.ap().rearrange("(c p) n -> p c n", p=128))
        nc.sync.dma_start(
            out=wk_sb, in_=wk_d.ap().rearrange("(c p) n -> p c n", p=128))
        nc.sync.dma_start(
            out=wv_sb, in_=wv_d.ap().rearrange("(c p) n -> p c n", p=128))

        for _ in range(n_iters):
            # per-head persistent activations
            qt_sb = [persist.tile([128, S], bf16, tag=f"qt{m}", name=f"qt{m}")
                     for m in range(HP)]
            kt_sb = [persist.tile([128, S], bf16, tag=f"kt{m}", name=f"kt{m}")
                     for m in range(HP)]
            v_sb = persist.tile([128, KT, C], bf16, tag="v", name="v")
            ot_sb = [persist.tile([128, S], bf16, tag=f"ot{m}", name=f"ot{m}")
                     for m in range(HP)]

            # ================= Phase 1: projections =================
            # QT / KT: [C, S] = W.T @ X.T, in (m, n-pair) chains of 2 banks;
            # n descending so drains match rev-j attention consumption.
            for tname, xdram, wsb, qkts, bcol in (
                ("q", xq_t, wq_sb, qt_sb, 0),
                ("k", xk_t, wk_sb, kt_sb, 2),
            ):
                xcs = []
                for c in range(8):
                    xc = xt_pool.tile([128, S], bf16, tag="xtc",
                                      name=f"x{tname}{c}")
                    nc.sync.dma_start(
                        out=xc, in_=xdram[c * 128:(c + 1) * 128, :])
                    xcs.append(xc)
                nseq = ((3, 2, 1, 0) if o["rev_j"] else (0, 1, 2, 3))
                if o["qk_chain"] == "single":
                    # one PSUM bank per (m, n) chain: drain of bank A hides
                    # under bank B's matmuls (true double buffering)
                    for n in nseq:
                        for m in range(HP):
                            ps = ps_proj.tile([128, 512], f32,
                                              tag="ps_proj",
                                              name=f"ps_{tname}{m}{n}")
                            for c in range(8):
                                nc.tensor.matmul(
                                    ps,
                                    lhsT=wsb[:, c, m * 128:(m + 1) * 128],
                                    rhs=xcs[c][:, n * 512:(n + 1) * 512],
                                    start=(c == 0), stop=(c == 7))
                            if o["qk_drain"] == "act":
                                nc.scalar.activation(
                                    out=qkts[m][:, n * 512:(n + 1) * 512],
                                    in_=ps, func=AF.Identity,
                                    bias=bqk[:, bcol + m:bcol + m + 1],
                                    scale=1.0)
                            else:
                                deng = (nc.gpsimd
                                        if o["qk_drain"] == "pool"
                                        else nc.vector)
                                deng.tensor_scalar_add(
                                    qkts[m][:, n * 512:(n + 1) * 512],
                                    ps, bqk[:, bcol + m:bcol + m + 1])
                else:
                    npairs = ((nseq[0], nseq[1]), (nseq[2], nseq[3]))
                    for npair in npairs:
                        for m in range(HP):
                            pss = {n: ps_proj.tile([128, 512], f32,
                                                   tag="ps_proj",
                                                   name=f"ps_{tname}{m}{n}")
                                   for n in npair}
                            for c in range(8):
                                for n in npair:
                                    nc.tensor.matmul(
                                        pss[n],
                                        lhsT=wsb[:, c, m * 128:(m + 1) * 128],
                                        rhs=xcs[c][:, n * 512:(n + 1) * 512],
                                        start=(c == 0), stop=(c == 7))
                            for n in npair:
                                deng = (nc.gpsimd if o["qk_drain"] == "pool"
                                        else nc.vector)
                                deng.tensor_scalar_add(
                                    qkts[m][:, n * 512:(n + 1) * 512],
                                    pss[n],
                                    bqk[:, bcol + m:bcol + m + 1])

            # V natural: [S, C] = X @ Wv (lhsT = XT chunk slice), s descending
            xcs = []
            for c in range(8):
                xc = xt_pool.tile([128, S], bf16, tag="xtc", name=f"xv{c}")
                nc.sync.dma_start(out=xc, in_=xv_t[c * 128:(c + 1) * 128, :])
                xcs.append(xc)
            vseq = range(KT - 1, -1, -1) if o["rev_j"] else range(KT)
            for s in vseq:
                psf = ps_proj.tile([128, 512], f32, tag="ps_proj",
                                   name=f"ps_v{s}")
                ps = psf[:, 0:C]
                for c in range(8):
                    nc.tensor.matmul(
                        ps,
                        lhsT=xcs[c][:, s * 128:(s + 1) * 128],
                        rhs=wv_sb[:, c, :],
                        start=(c == 0), stop=(c == 7))
                if o["v_drain"] == "pool":
                    nc.gpsimd.tensor_copy(v_sb[:, s, :], ps)
                elif o["v_drain"] == "dve":
                    nc.vector.tensor_copy(v_sb[:, s, :], ps)
                else:
                    nc.scalar.copy(v_sb[:, s, :], ps)

            # ================= Phase 2: attention (per head) =================
            jseq = (list(reversed(range(QC))) if o["rev_j"]
                    else list(range(QC)))
            for j in jseq:
                nk = 4 * (j + 1)
                ng = nk // 4
                qsl = slice(j * 512, (j + 1) * 512)
                hseq = ((0, 1) if (j % 2 == 0 or not o["hj_alt"])
                        else (1, 0))
                for h in hseq:
                    et = expt_pool.tile([128, KT, 512], bf16, tag="et",
                                        name=f"et{h}{j}")
                    for kti in range(nk):
                        diag = kti >= 4 * j
                        ps = ps_sc.tile([128, 512], f32, tag="ps_sc",
                                        name=f"sc{h}{j}{kti}")
                        ktile = kt_sb[h][:, kti * 128:(kti + 1) * 128]
                        if not (diag and o["slim_diag"]):
                            if diag:
                                nc.tensor.matmul(ps, lhsT=ident, rhs=tri_sb,
                                                 start=True, stop=False)
                                # full-width fallback needs mask everywhere;
                                # only used when slim_diag is off
                                raise NotImplementedError(
                                    "slim_diag=False not supported")
                            nc.tensor.matmul(
                                ps, lhsT=ktile, rhs=qt_sb[h][:, qsl],
                                start=True, stop=True)
                            nc.scalar.activation(out=et[:, kti, :], in_=ps,
                                                 func=AF.Exp, scale=SCALE)
                        elif o["tri_sel"]:
                            i = kti - 4 * j
                            c0 = 128 * i
                            nc.tensor.matmul(
                                ps[:, c0:512], lhsT=ktile,
                                rhs=qt_sb[h][:, j * 512 + c0:(j + 1) * 512],
                                start=True, stop=True)
                            nc.scalar.activation(out=et[:, kti, c0:512],
                                                 in_=ps[:, c0:512],
                                                 func=AF.Exp, scale=SCALE)
                            # zero the strictly-upper triangle (k > q) of
                            # the diagonal 128x128 sub-block, post-exp
                            nc.gpsimd.affine_select(
                                out=et[:, kti, c0:c0 + 128],
                                in_=et[:, kti, c0:c0 + 128],
                                compare_op=mybir.AluOpType.is_ge,
                                fill=0.0, base=0,
                                pattern=[[1, 128]], channel_multiplier=-1)
                        else:
                            i = kti - 4 * j
                            c0 = 128 * i
                            nc.tensor.matmul(ps[:, c0:c0 + 128], lhsT=ident,
                                             rhs=tri_sb, start=True,
                                             stop=False)
                            nc.tensor.matmul(
                                ps[:, c0:c0 + 128], lhsT=ktile,
                                rhs=qt_sb[h][:, j * 512 + c0:
                                             j * 512 + c0 + 128],
                                start=False, stop=True)
                            if c0 + 128 < 512:
                                nc.tensor.matmul(
                                    ps[:, c0 + 128:512], lhsT=ktile,
                                    rhs=qt_sb[h][:, j * 512 + c0 + 128:
                                                 (j + 1) * 512],
                                    start=True, stop=True)
                            nc.scalar.activation(out=et[:, kti, c0:512],
                                                 in_=ps[:, c0:512],
                                                 func=AF.Exp, scale=SCALE)

                    # attn @ V -> outT[d, qchunk] (accumulate over k-tiles)
                    ot = ps_ot.tile([128, 512], f32, tag="ps_ot",
                                    name=f"ot{h}{j}")
                    for kti in range(nk):
                        diag = kti >= 4 * j
                        c0 = 128 * (kti - 4 * j) if (diag and o["slim_diag"]) \
                            else 0
                        nc.tensor.matmul(
                            ot[:, c0:512],
                            lhsT=v_sb[:, kti, h * 128:(h + 1) * 128],
                            rhs=et[:, kti, c0:512],
                            start=(kti == 0), stop=(kti == nk - 1))

                    # column sums: non-diag groups as bf16 partials on
                    # DVE (h0) / Pool (h1); diagonal group summed on PE
                    # directly into the cs PSUM (removes the serial
                    # vector-engine tail before the reciprocal).
                    parts = []
                    for g in range(j):
                        pg = part_pool.tile([128, 512], bf16, tag="part",
                                            name=f"pt{h}{j}{g}")
                        if o["partials"] == "pairwise":
                            a0 = part_pool.tile([128, 512], bf16,
                                                tag="part",
                                                name=f"pa{h}{j}{g}")
                            eng = (nc.gpsimd if o["part_pool_eng"]
                                   else nc.vector)
                            eng.tensor_add(a0, et[:, 4 * g, :],
                                           et[:, 4 * g + 1, :])
                            nc.vector.tensor_add(pg, et[:, 4 * g + 2, :],
                                                 et[:, 4 * g + 3, :])
                            nc.vector.tensor_add(pg, pg, a0)
                        elif h == 0:
                            with nc.allow_low_precision(
                                    reason="softmax partials bf16"):
                                nc.vector.tensor_reduce(
                                    out=pg,
                                    in_=et[:, 4 * g:4 * g + 4, :]
                                    .rearrange("p k q -> p q k"),
                                    axis=mybir.AxisListType.X,
                                    op=mybir.AluOpType.add)
                        else:
                            a0 = part_pool.tile([128, 512], bf16,
                                                tag="part",
                                                name=f"pa{h}{j}{g}")
                            nc.gpsimd.tensor_add(a0, et[:, 4 * g, :],
                                                 et[:, 4 * g + 1, :])
                            nc.gpsimd.tensor_add(pg, et[:, 4 * g + 2, :],
                                                 et[:, 4 * g + 3, :])
                            nc.gpsimd.tensor_add(pg, pg, a0)
                        parts.append(pg)
                    accum = None
                    if j == 1:
                        accum = parts[0]
                    elif j >= 2:
                        accum = acc_pool.tile([128, 512], bf16, tag="accum",
                                              name=f"ac{h}{j}")
                        nc.vector.tensor_add(accum, parts[0], parts[1])
                        for g in range(2, j):
                            nc.vector.tensor_add(accum, accum, parts[g])

                    csf = ps_csop.tile([128, 512], f32, tag="ps_csop",
                                       name=f"cs{h}{j}")
                    cs = csf[0:1, :]
                    if o["dve_diag_sum"]:
                        # diagonal group summed piecewise on DVE (valid
                        # columns only), then a single ones matmul
                        dg = part_pool.tile([128, 512], bf16, tag="part",
                                            name=f"dg{h}{j}")
                        nc.vector.tensor_copy(dg, et[:, 4 * j, :])
                        for i in range(1, 4):
                            c0 = 128 * i
                            nc.vector.tensor_add(dg[:, c0:512],
                                                 dg[:, c0:512],
                                                 et[:, 4 * j + i, c0:512])
                        if accum is not None:
                            acc2 = acc_pool.tile([128, 512], bf16,
                                                 tag="accum",
                                                 name=f"ac2{h}{j}")
                            nc.vector.tensor_add(acc2, accum, dg)
                            accum = acc2
                        else:
                            accum = dg
                        nc.tensor.matmul(cs, lhsT=ones_col_bf, rhs=accum,
                                         start=True, stop=True)
                    else:
                        # cs[1,512] = ones @ (accum + diag-group et tiles)
                        nc.tensor.matmul(cs, lhsT=ones_col_bf,
                                         rhs=et[:, 4 * j, :],
                                         start=True, stop=False)
                        for i in range(1, 4):
                            c0 = 128 * i
                            last = (i == 3) and accum is None
                            nc.tensor.matmul(cs[:, c0:512], lhsT=ones_col_bf,
                                             rhs=et[:, 4 * j + i, c0:512],
                                             start=False, stop=last)
                        if accum is not None:
                            nc.tensor.matmul(cs, lhsT=ones_col_bf, rhs=accum,
                                             start=False, stop=True)
                    rinv = acc_pool.tile([1, 512], bf16, tag="rinv",
                                         name=f"ri{h}{j}")
                    with nc.allow_low_precision(
                            reason="softmax reciprocal scale bf16"):
                        nc.vector.reciprocal(rinv, cs)
                    rs_ps = ps_csop.tile([128, 512], f32, tag="ps_csop",
                                         name=f"rs{h}{j}")
                    nc.tensor.matmul(rs_ps, lhsT=ones_row_bf, rhs=rinv,
                                     start=True, stop=True)
                    if o["fuse_rs"]:
                        nc.vector.tensor_mul(ot_sb[h][:, qsl], ot, rs_ps)
                    else:
                        rs_sb = acc_pool.tile([128, 512], f32, tag="rssb",
                                              name=f"rb{h}{j}")
                        nc.vector.tensor_copy(rs_sb, rs_ps)
                        nc.vector.tensor_mul(ot_sb[h][:, qsl], ot, rs_sb)

                # fused out-projection for this q-block (both heads done)
                for s in range(4 * j, 4 * j + 4):
                    osb = outst.tile([128, E], bf16, tag="osb",
                                     name=f"osb{s}")
                    if o["op_reorder"]:
                        # head-outer so each ot stationary is loaded once
                        pss = [ps_csop.tile([128, 512], f32, tag="ps_csop",
                                            name=f"op{s}{nch}")
                               for nch in range(2)]
                        for hh in range(HP):
                            for nch in range(2):
                                nsl = slice(nch * 512, (nch + 1) * 512)
                                nc.tensor.matmul(
                                    pss[nch],
                                    lhsT=ot_sb[hh][:, s * 128:(s + 1) * 128],
                                    rhs=wo_sb[:, hh, nsl],
                                    start=(hh == 0), stop=(hh == HP - 1))
                        nc.vector.tensor_copy(osb[:, 0:512], pss[0])
                        nc.scalar.copy(osb[:, 512:1024], pss[1])
                    else:
                        for nch in range(2):
                            nsl = slice(nch * 512, (nch + 1) * 512)
                            ps = ps_csop.tile([128, 512], f32, tag="ps_csop",
                                              name=f"op{s}{nch}")
                            for hh in range(HP):
                                nc.tensor.matmul(
                                    ps,
                                    lhsT=ot_sb[hh][:, s * 128:(s + 1) * 128],
                                    rhs=wo_sb[:, hh, nsl],
                                    start=(hh == 0), stop=(hh == HP - 1))
                            if o["osb_split"] == "pool" and nch == 1:
                                nc.gpsimd.tensor_copy(osb[:, nsl], ps)
                            elif o["osb_split"] and nch == 1:
                                nc.scalar.copy(osb[:, nsl], ps)
                            elif o["osb0_pool"]:
                                nc.gpsimd.tensor_copy(osb[:, nsl], ps)
                            else:
                                nc.vector.tensor_copy(osb[:, nsl], ps)
                    (nc.sync if o["out_sp"] else nc.gpsimd).dma_start(
                        out=out_d[s * 128:(s + 1) * 128, :], in_=osb)

    nc.compile()
    return nc


def get_program(n_iters: int = 1):
    if n_iters not in _prog_cache:
        _prog_cache[n_iters] = build_program(n_iters)
    return _prog_cache[n_iters]


def make_in_maps(query, key_, value, Wq, bq, Wk, bk, Wv, bv, Wo, bo, mask):
    """Host-side sharding: build the 8 per-core input maps."""
    query = np.asarray(query, np.float32)
    key_ = np.asarray(key_, np.float32)
    value = np.asarray(value, np.float32)
    mask = np.asarray(mask)

    # transposed bf16 activations per batch: [E, S]
    xt = {}
    for b in range(B):
        xt[("q", b)] = np.ascontiguousarray(query[b].T.astype(BF16))
        xt[("k", b)] = np.ascontiguousarray(key_[b].T.astype(BF16))
        xt[("v", b)] = np.ascontiguousarray(value[b].T.astype(BF16))

    # additive transposed triangular mask for the diagonal 128x128 blocks
    # (identical for every diagonal-straddling block of a causal mask)
    m2 = np.asarray(mask).reshape(S, S)
    blk = m2[0:128, 0:128]                       # [q, k]
    tri = np.where(blk.T != 0, 0.0, MASK_NEG)    # [k, q]
    # additive mask is applied pre-scale, so divide by SCALE
    tri = (tri / SCALE).astype(BF16)

    Wq = np.asarray(Wq, np.float32)
    Wk = np.asarray(Wk, np.float32)
    Wv = np.asarray(Wv, np.float32)
    Wo = np.asarray(Wo, np.float32)
    bq = np.asarray(bq, np.float32)
    bk = np.asarray(bk, np.float32)

    in_maps = []
    for c in range(NCORES):
        b, g = divmod(c, 4)
        c0 = C * g
        bqk = np.stack([bq[c0:c0 + 128], bq[c0 + 128:c0 + 256],
                        bk[c0:c0 + 128], bk[c0 + 128:c0 + 256]], axis=1)
        in_maps.append({
            "xq_t": xt[("q", b)],
            "xk_t": xt[("k", b)],
            "xv_t": xt[("v", b)],
            "wq": Wq[:, c0:c0 + C].astype(BF16),
            "wk": Wk[:, c0:c0 + C].astype(BF16),
            "wv": Wv[:, c0:c0 + C].astype(BF16),
            "wo": np.ascontiguousarray(Wo[c0:c0 + C, :]).astype(BF16),
            "bqk": np.ascontiguousarray(bqk, dtype=np.float32),
            "tri": tri,
        })
    return in_maps


def gather_output(results, Wo, bv, bo):
    out = np.zeros((B, S, E), np.float32)
    for c in range(NCORES):
        b = c // 4
        out[b] += np.asarray(results[c]["out"], np.float32)
    # V-bias contributes + bv @ Wo to every row (softmax weights sum to 1)
    out += (np.asarray(bo, np.float32)
            + np.asarray(bv, np.float32) @ np.asarray(Wo, np.float32))
    return out


def kernel(**inputs) -> np.ndarray:
    from concourse.bass_utils import run_bass_kernel_spmd

    nc = get_program(1)
    in_maps = make_in_maps(**inputs)
    res = run_bass_kernel_spmd(nc, in_maps, core_ids=list(range(NCORES)))
    return gather_output(res.results, inputs["Wo"], inputs["bv"],
                         inputs["bo"])



# revision 1
# speedup vs baseline: 1.1656x; 1.1656x over previous
"""Trainium2 Bass kernel for nn_MultiHeadAttention (B=2, S=2048, E=1024, H=8, D=128).

Sharding (8 cores): core c handles batch b=c//4 and head-pair g=c%4
(heads 2g, 2g+1 -> E-columns [256g, 256g+256)).
 - Q/K/V projections column-parallel (each core computes its 256 columns).
 - Attention device-local per head, computed in transposed score layout
   scoresT[k, q] so softmaxed weights are directly the rhs of attn@V.
 - Out-projection row-parallel: each core produces a full-shape partial
   out_partial = attn_out_heads @ Wo[rows] in bf16; host sums 4 partials
   per batch and adds the bo / bv-induced bias row (softmax weights sum
   to 1, so the V bias contributes exactly + bv @ Wo per output row).
 - Causal structure: fully-masked (strictly upper) 128x512 blocks are
   skipped; diagonal-straddling 128-wide sub-blocks get the (identical)
   triangular additive mask via a narrow identity matmul; scores/exp/
   attn@V are restricted to the valid column range.
 - PSUM pools are persistent and split (proj 2 / scores 2 / attn-out 2 /
   shared colsum+broadcast+outproj 2 banks) so the tile scheduler can
   overlap iteration i+1's projections with iteration i's attention;
   projection chains use one bank each (drains double-buffer) and are
   ordered to match the attention q-chunk order.
 - Softmax row-sums: non-diagonal k-tile groups pairwise-added to bf16
   partials (Pool first pair, DVE rest); the diagonal group is summed
   piecewise on DVE (valid columns only) and folded into the accumulator,
   leaving a single ones-matmul on the PE for the partition reduce.
"""

import os
import sys

for _p in ("/opt/trn_rl_repo", os.environ.get("TRN_RL_REPO", "")):
    if _p and os.path.isdir(_p) and _p not in sys.path:
        sys.path.insert(0, _p)

import numpy as np
import ml_dtypes

BF16 = ml_dtypes.bfloat16

B, S, E, H = 2, 2048, 1024, 8
D = E // H          # 128
HP = 2              # heads per core
C = HP * D          # 256 projection columns per core
NCORES = 8
KT = S // 128       # 16 k-tiles
QC = S // 512       # 4 q-chunks
SCALE = 1.0 / float(np.sqrt(D))
MASK_NEG = -30000.0

_prog_cache = {}


def build_program(n_iters: int = 1, **opt):
    """Build the SPMD Bass program (Tile). Returns the compiled Bacc object."""
    import concourse.bass as bass
    import concourse.mybir as mybir
    import concourse.tile as tile
    from concourse import bacc, bass_isa
    from concourse.masks import make_identity
    from contextlib import ExitStack

    f32 = mybir.dt.float32
    bf16 = mybir.dt.bfloat16
    AF = mybir.ActivationFunctionType

    o = dict(xt_bufs=16, expt_bufs=2, part_bufs=5, acc_bufs=3, outst_bufs=4,
             proj_bufs=2, sc_bufs=2, ot_bufs=2, csop_bufs=2,
             slim_diag=True, pair_reduce=False, rev_j=False, osb_split=False,
             fuse_rs=False, v_drain="act", tri_sel=False,
             part_pool_eng=True, dve_diag_sum=True,
             partials="pairwise", qk_drain="dve", osb0_pool=False, op_reorder=False, qk_chain="single", hj_alt=False, out_sp=False)
    o.update(opt)

    nc = bacc.Bacc("TRN2", target_bir_lowering=False, debug=False,
                   enable_partition_id=False)

    # ---- DRAM I/O (per-core slices supplied by the host) ----
    xq_t = nc.dram_tensor("xq_t", [E, S], bf16, kind="ExternalInput")
    xk_t = nc.dram_tensor("xk_t", [E, S], bf16, kind="ExternalInput")
    xv_t = nc.dram_tensor("xv_t", [E, S], bf16, kind="ExternalInput")
    wq_d = nc.dram_tensor("wq", [E, C], bf16, kind="ExternalInput")
    wk_d = nc.dram_tensor("wk", [E, C], bf16, kind="ExternalInput")
    wv_d = nc.dram_tensor("wv", [E, C], bf16, kind="ExternalInput")
    wo_d = nc.dram_tensor("wo", [C, E], bf16, kind="ExternalInput")
    bqk_d = nc.dram_tensor("bqk", [128, 4], f32, kind="ExternalInput")
    tri_d = nc.dram_tensor("tri", [128, 128], bf16, kind="ExternalInput")
    out_d = nc.dram_tensor("out", [S, E], bf16, kind="ExternalOutput")

    with tile.TileContext(nc) as tc, ExitStack() as ctx:
        persist = ctx.enter_context(tc.tile_pool(name="persist", bufs=1))
        xt_pool = ctx.enter_context(tc.tile_pool(name="xt", bufs=o["xt_bufs"]))
        expt_pool = ctx.enter_context(tc.tile_pool(name="expt",
                                                   bufs=o["expt_bufs"]))
        acc_pool = ctx.enter_context(tc.tile_pool(name="acc",
                                                  bufs=o["acc_bufs"]))
        part_pool = ctx.enter_context(tc.tile_pool(name="part",
                                                   bufs=o["part_bufs"]))
        outst = ctx.enter_context(tc.tile_pool(name="outst",
                                               bufs=o["outst_bufs"]))
        # persistent PSUM pools: 2+2+2+2 = 8 banks
        ps_proj = ctx.enter_context(
            tc.tile_pool(name="ps_proj", bufs=o["proj_bufs"], space="PSUM"))
        ps_sc = ctx.enter_context(
            tc.tile_pool(name="ps_sc", bufs=o["sc_bufs"], space="PSUM"))
        ps_ot = ctx.enter_context(
            tc.tile_pool(name="ps_ot", bufs=o["ot_bufs"], space="PSUM"))
        ps_csop = ctx.enter_context(
            tc.tile_pool(name="ps_csop", bufs=o["csop_bufs"], space="PSUM"))

        # ---- constants ----
        ident = persist.tile([128, 128], bf16, tag="ident")
        make_identity(nc, ident)
        ones_col = persist.tile([128, 1], f32, tag="ones_col")
        nc.vector.memset(ones_col, 1.0)
        ones_row = persist.tile([1, 128], f32, tag="ones_row")
        nc.vector.memset(ones_row, 1.0)
        ones_col_bf = persist.tile([128, 1], bf16, tag="ones_col_bf")
        nc.vector.memset(ones_col_bf, 1.0)
        ones_row_bf = persist.tile([1, 128], bf16, tag="ones_row_bf")
        nc.vector.memset(ones_row_bf, 1.0)

        # ---- persistent weight / bias / mask tiles ----
        wq_sb = persist.tile([128, 8, C], bf16, tag="wq")
        wk_sb = persist.tile([128, 8, C], bf16, tag="wk")
        wv_sb = persist.tile([128, 8, C], bf16, tag="wv")
        wo_sb = persist.tile([128, HP, E], bf16, tag="wo")
        bqk = persist.tile([128, 4], f32, tag="bqk")
        tri_sb = persist.tile([128, 128], bf16, tag="tri")

        # late-needed loads on the SWDGE queue so they don't block the
        # activation stream on the HWDGE queue
        nc.gpsimd.dma_start(out=tri_sb, in_=tri_d.ap())
        nc.gpsimd.dma_start(out=wo_sb,
                            in_=wo_d.ap().rearrange("(h p) n -> p h n", p=128))
        nc.sync.dma_start(out=bqk, in_=bqk_d.ap())
        nc.sync.dma_start(
            out=wq_sb, in_=wq_d.ap().rearrange("(c p) n -> p c n", p=128))
        nc.sync.dma_start(
            out=wk_sb, in_=wk_d.ap().rearrange("(c p) n -> p c n", p=128))
        nc.sync.dma_start(
            out=wv_sb, in_=wv_d.ap().rearrange("(c p) n -> p c n", p=128))

        for _ in range(n_iters):
            # per-head persistent activations
            qt_sb = [persist.tile([128, S], bf16, tag=f"qt{m}", name=f"qt{m}")
                     for m in range(HP)]
            kt_sb = [persist.tile([128, S], bf16, tag=f"kt{m}", name=f"kt{m}")
                     for m in range(HP)]
            v_sb = persist.tile([128, KT, C], bf16, tag="v", name="v")
            ot_sb = [persist.tile([128, S], bf16, tag=f"ot{m}", name=f"ot{m}")
                     for m in range(HP)]

            # ================= Phase 1: projections =================
            # QT / KT: [C, S] = W.T @ X.T, in (m, n-pair) chains of 2 banks;
            # n descending so drains match rev-j attention consumption.
            for tname, xdram, wsb, qkts, bcol in (
                ("q", xq_t, wq_sb, qt_sb, 0),
                ("k", xk_t, wk_sb, kt_sb, 2),
            ):
                xcs = []
                for c in range(8):
                    xc = xt_pool.tile([128, S], bf16, tag="xtc",
                                      name=f"x{tname}{c}")
                    nc.sync.dma_start(
                        out=xc, in_=xdram[c * 128:(c + 1) * 128, :])
                    xcs.append(xc)
                nseq = ((3, 2, 1, 0) if o["rev_j"] else (0, 1, 2, 3))
                if o["qk_chain"] == "single":
                    # one PSUM bank per (m, n) chain: drain of bank A hides
                    # under bank B's matmuls (true double buffering)
                    for n in nseq:
                        for m in range(HP):
                            ps = ps_proj.tile([128, 512], f32,
                                              tag="ps_proj",
                                              name=f"ps_{tname}{m}{n}")
                            for c in range(8):
                                nc.tensor.matmul(
                                    ps,
                                    lhsT=wsb[:, c, m * 128:(m + 1) * 128],
                                    rhs=xcs[c][:, n * 512:(n + 1) * 512],
                                    start=(c == 0), stop=(c == 7))
                            if o["qk_drain"] == "act":
                                nc.scalar.activation(
                                    out=qkts[m][:, n * 512:(n + 1) * 512],
                                    in_=ps, func=AF.Identity,
                                    bias=bqk[:, bcol + m:bcol + m + 1],
                                    scale=1.0)
                            else:
                                deng = (nc.gpsimd
                                        if o["qk_drain"] == "pool"
                                        else nc.vector)
                                deng.tensor_scalar_add(
                                    qkts[m][:, n * 512:(n + 1) * 512],
                                    ps, bqk[:, bcol + m:bcol + m + 1])
                else:
                    npairs = ((nseq[0], nseq[1]), (nseq[2], nseq[3]))
                    for npair in npairs:
                        for m in range(HP):
                            pss = {n: ps_proj.tile([128, 512], f32,
                                                   tag="ps_proj",
                                                   name=f"ps_{tname}{m}{n}")
                                   for n in npair}
                            for c in range(8):
                                for n in npair:
                                    nc.tensor.matmul(
                                        pss[n],
                                        lhsT=wsb[:, c, m * 128:(m + 1) * 128],
                                        rhs=xcs[c][:, n * 512:(n + 1) * 512],
                                        start=(c == 0), stop=(c == 7))
                            for n in npair:
                                deng = (nc.gpsimd if o["qk_drain"] == "pool"
                                        else nc.vector)
                                deng.tensor_scalar_add(
                                    qkts[m][:, n * 512:(n + 1) * 512],
                                    pss[n],
                                    bqk[:, bcol + m:bcol + m + 1])

            # V natural: [S, C] = X @ Wv (lhsT = XT chunk slice), s descending
            xcs = []
            for c in range(8):
                xc = xt_pool.tile([128, S], bf16, tag="xtc", name=f"xv{c}")
                nc.sync.dma_start(out=xc, in_=xv_t[c * 128:(c + 1) * 128, :])
                xcs.append(xc)
            vseq = range(KT - 1, -1, -1) if o["rev_j"] else range(KT)
            for s in vseq:
                psf = ps_proj.tile([128, 512], f32, tag="ps_proj",
                                   name=f"ps_v{s}")
                ps = psf[:, 0:C]
                for c in range(8):
                    nc.tensor.matmul(
                        ps,
                        lhsT=xcs[c][:, s * 128:(s + 1) * 128],
                        rhs=wv_sb[:, c, :],
                        start=(c == 0), stop=(c == 7))
                if o["v_drain"] == "pool":
                    nc.gpsimd.tensor_copy(v_sb[:, s, :], ps)
                elif o["v_drain"] == "dve":
                    nc.vector.tensor_copy(v_sb[:, s, :], ps)
                else:
                    nc.scalar.copy(v_sb[:, s, :], ps)

            # ================= Phase 2: attention (per head) =================
            jseq = (list(reversed(range(QC))) if o["rev_j"]
                    else list(range(QC)))
            for j in jseq:
                nk = 4 * (j + 1)
                ng = nk // 4
                qsl = slice(j * 512, (j + 1) * 512)
                hseq = ((0, 1) if (j % 2 == 0 or not o["hj_alt"])
                        else (1, 0))
                for h in hseq:
                    et = expt_pool.tile([128, KT, 512], bf16, tag="et",
                                        name=f"et{h}{j}")
                    for kti in range(nk):
                        diag = kti >= 4 * j
                        ps = ps_sc.tile([128, 512], f32, tag="ps_sc",
                                        name=f"sc{h}{j}{kti}")
                        ktile = kt_sb[h][:, kti * 128:(kti + 1) * 128]
                        if not (diag and o["slim_diag"]):
                            if diag:
                                nc.tensor.matmul(ps, lhsT=ident, rhs=tri_sb,
                                                 start=True, stop=False)
                                # full-width fallback needs mask everywhere;
                                # only used when slim_diag is off
                                raise NotImplementedError(
                                    "slim_diag=False not supported")
                            nc.tensor.matmul(
                                ps, lhsT=ktile, rhs=qt_sb[h][:, qsl],
                                start=True, stop=True)
                            nc.scalar.activation(out=et[:, kti, :], in_=ps,
                                                 func=AF.Exp, scale=SCALE)
                        elif o["tri_sel"]:
                            i = kti - 4 * j
                            c0 = 128 * i
                            nc.tensor.matmul(
                                ps[:, c0:512], lhsT=ktile,
                                rhs=qt_sb[h][:, j * 512 + c0:(j + 1) * 512],
                                start=True, stop=True)
                            nc.scalar.activation(out=et[:, kti, c0:512],
                                                 in_=ps[:, c0:512],
                                                 func=AF.Exp, scale=SCALE)
                            # zero the strictly-upper triangle (k > q) of
                            # the diagonal 128x128 sub-block, post-exp
                            nc.gpsimd.affine_select(
                                out=et[:, kti, c0:c0 + 128],
                                in_=et[:, kti, c0:c0 + 128],
                                compare_op=mybir.AluOpType.is_ge,
                                fill=0.0, base=0,
                                pattern=[[1, 128]], channel_multiplier=-1)
                        else:
                            i = kti - 4 * j
                            c0 = 128 * i
                            nc.tensor.matmul(ps[:, c0:c0 + 128], lhsT=ident,
                                             rhs=tri_sb, start=True,
                                             stop=False)
                            nc.tensor.matmul(
                                ps[:, c0:c0 + 128], lhsT=ktile,
                                rhs=qt_sb[h][:, j * 512 + c0:
                                             j * 512 + c0 + 128],
                                start=False, stop=True)
                            if c0 + 128 < 512:
                                nc.tensor.matmul(
                                    ps[:, c0 + 128:512], lhsT=ktile,
                                    rhs=qt_sb[h][:, j * 512 + c0 + 128:
                                                 (j + 1) * 512],
                                    start=True, stop=True)
                            nc.scalar.activation(out=et[:, kti, c0:512],
                                                 in_=ps[:, c0:512],
                                                 func=AF.Exp, scale=SCALE)

                    # attn @ V -> outT[d, qchunk] (accumulate over k-tiles)
                    ot = ps_ot.tile([128, 512], f32, tag="ps_ot",
                                    name=f"ot{h}{j}")
                    for kti in range(nk):
                        diag = kti >= 4 * j
                        c0 = 128 * (kti - 4 * j) if (diag and o["slim_diag"]) \
                            else 0
                        nc.tensor.matmul(
                            ot[:, c0:512],
                            lhsT=v_sb[:, kti, h * 128:(h + 1) * 128],
                            rhs=et[:, kti, c0:512],
                            start=(kti == 0), stop=(kti == nk - 1))

                    # column sums: non-diag groups as bf16 partials on
                    # DVE (h0) / Pool (h1); diagonal group summed on PE
                    # directly into the cs PSUM (removes the serial
                    # vector-engine tail before the reciprocal).
                    parts = []
                    for g in range(j):
                        pg = part_pool.tile([128, 512], bf16, tag="part",
                                            name=f"pt{h}{j}{g}")
                        if o["partials"] == "pairwise":
                            a0 = part_pool.tile([128, 512], bf16,
                                                tag="part",
                                                name=f"pa{h}{j}{g}")
                            eng = (nc.gpsimd if o["part_pool_eng"]
                                   else nc.vector)
                            eng.tensor_add(a0, et[:, 4 * g, :],
                                           et[:, 4 * g + 1, :])
                            nc.vector.tensor_add(pg, et[:, 4 * g + 2, :],
                                                 et[:, 4 * g + 3, :])
                            nc.vector.tensor_add(pg, pg, a0)
                        elif h == 0:
                            with nc.allow_low_precision(
                                    reason="softmax partials bf16"):
                                nc.vector.tensor_reduce(
                                    out=pg,
                                    in_=et[:, 4 * g:4 * g + 4, :]
                                    .rearrange("p k q -> p q k"),
                                    axis=mybir.AxisListType.X,
                                    op=mybir.AluOpType.add)
                        else:
                            a0 = part_pool.tile([128, 512], bf16,
                                                tag="part",
                                                name=f"pa{h}{j}{g}")
                            nc.gpsimd.tensor_add(a0, et[:, 4 * g, :],
                                                 et[:, 4 * g + 1, :])
                            nc.gpsimd.tensor_add(pg, et[:, 4 * g + 2, :],
                                                 et[:, 4 * g + 3, :])
                            nc.gpsimd.tensor_add(pg, pg, a0)
                        parts.append(pg)
                    accum = None
                    if j == 1:
                        accum = parts[0]
                    elif j >= 2:
                        accum = acc_pool.tile([128, 512], bf16, tag="accum",
                                              name=f"ac{h}{j}")
                        nc.vector.tensor_add(accum, parts[0], parts[1])
                        for g in range(2, j):
                            nc.vector.tensor_add(accum, accum, parts[g])

                    csf = ps_csop.tile([128, 512], f32, tag="ps_csop",
                                       name=f"cs{h}{j}")
                    cs = csf[0:1, :]
                    if o["dve_diag_sum"]:
                        # diagonal group summed piecewise on DVE (valid
                        # columns only), then a single ones matmul
                        dg = part_pool.tile([128, 512], bf16, tag="part",
                                            name=f"dg{h}{j}")
                        nc.vector.tensor_copy(dg, et[:, 4 * j, :])
                        for i in range(1, 4):
                            c0 = 128 * i
                            nc.vector.tensor_add(dg[:, c0:512],
                                                 dg[:, c0:512],
                                                 et[:, 4 * j + i, c0:512])
                        if accum is not None:
                            acc2 = acc_pool.tile([128, 512], bf16,
                                                 tag="accum",
                                                 name=f"ac2{h}{j}")
                            nc.vector.tensor_add(acc2, accum, dg)
                            accum = acc2
                        else:
                            accum = dg
                        nc.tensor.matmul(cs, lhsT=ones_col_bf, rhs=accum,
                                         start=True, stop=True)
                    else:
                        # cs[1,512] = ones @ (accum + diag-group et tiles)
                        nc.tensor.matmul(cs, lhsT=ones_col_bf,
                                         rhs=et[:, 4 * j, :],
                                         start=True, stop=False)
                        for i in range(1, 4):
                            c0 = 128 * i
                            last = (i == 3) and accum is None
                            nc.tensor.matmul(cs[:, c0:512], lhsT=ones_col_bf,
                                             rhs=et[:, 4 * j + i, c0:512],
                                             start=False, stop=last)
                        if accum is not None:
                            nc.tensor.matmul(cs, lhsT=ones_col_bf, rhs=accum,
                                             start=False, stop=True)
                    rinv = acc_pool.tile([1, 512], bf16, tag="rinv",
                                         name=f"ri{h}{j}")
                    with nc.allow_low_precision(
                            reason="softmax reciprocal scale bf16"):
                        nc.vector.reciprocal(rinv, cs)
                    rs_ps = ps_csop.tile([128, 512], f32, tag="ps_csop",
                                         name=f"rs{h}{j}")
                    nc.tensor.matmul(rs_ps, lhsT=ones_row_bf, rhs=rinv,
                                     start=True, stop=True)
                    if o["fuse_rs"]:
                        nc.vector.tensor_mul(ot_sb[h][:, qsl], ot, rs_ps)
                    else:
                        rs_sb = acc_pool.tile([128, 512], f32, tag="rssb",
                                              name=f"rb{h}{j}")
                        nc.vector.tensor_copy(rs_sb, rs_ps)
                        nc.vector.tensor_mul(ot_sb[h][:, qsl], ot, rs_sb)

                # fused out-projection for this q-block (both heads done)
                for s in range(4 * j, 4 * j + 4):
                    osb = outst.tile([128, E], bf16, tag="osb",
                                     name=f"osb{s}")
                    if o["op_reorder"]:
                        # head-outer so each ot stationary is loaded once
                        pss = [ps_csop.tile([128, 512], f32, tag="ps_csop",
                                            name=f"op{s}{nch}")
                               for nch in range(2)]
                        for hh in range(HP):
                            for nch in range(2):
                                nsl = slice(nch * 512, (nch + 1) * 512)
                                nc.tensor.matmul(
                                    pss[nch],
                                    lhsT=ot_sb[hh][:, s * 128:(s + 1) * 128],
                                    rhs=wo_sb[:, hh, nsl],
                                    start=(hh == 0), stop=(hh == HP - 1))
                        nc.vector.tensor_copy(osb[:, 0:512], pss[0])
                        nc.scalar.copy(osb[:, 512:1024], pss[1])
                    else:
                        for nch in range(2):
                            nsl = slice(nch * 512, (nch + 1) * 512)
                            ps = ps_csop.tile([128, 512], f32, tag="ps_csop",
                                              name=f"op{s}{nch}")
                            for hh in range(HP):
                                nc.tensor.matmul(
                                    ps,
                                    lhsT=ot_sb[hh][:, s * 128:(s + 1) * 128],
                                    rhs=wo_sb[:, hh, nsl],
                                    start=(hh == 0), stop=(hh == HP - 1))
                            if o["osb_split"] == "pool" and nch == 1:
                                nc.gpsimd.tensor_copy(osb[:, nsl], ps)
                            elif o["osb_split"] and nch == 1:
                                nc.scalar.copy(osb[:, nsl], ps)
                            elif o["osb0_pool"]:
                                nc.gpsimd.tensor_copy(osb[:, nsl], ps)
                            else:
                                nc.vector.tensor_copy(osb[:, nsl], ps)
                    (nc.sync if o["out_sp"] else nc.gpsimd).dma_start(
                        out=out_d[s * 128:(s + 1) * 128, :], in_=osb)

    nc.compile()
    return nc


def get_program(n_iters: int = 1):
    if n_iters not in _prog_cache:
        _prog_cache[n_iters] = build_program(n_iters)
    return _prog_cache[n_iters]


def make_in_maps(query, key_, value, Wq, bq, Wk, bk, Wv, bv, Wo, bo, mask):
    """Host-side sharding: build the 8 per-core input maps."""
    query = np.asarray(query, np.float32)
    key_ = np.asarray(key_, np.float32)
    value = np.asarray(value, np.float32)
    mask = np.asarray(mask)

    # transposed bf16 activations per batch: [E, S]
    xt = {}
    for b in range(B):
        xt[("q", b)] = np.ascontiguousarray(query[b].T.astype(BF16))
        xt[("k", b)] = np.ascontiguousarray(key_[b].T.astype(BF16))
        xt[("v", b)] = np.ascontiguousarray(value[b].T.astype(BF16))

    # additive transposed triangular mask for the diagonal 128x128 blocks
    # (identical for every diagonal-straddling block of a causal mask)
    m2 = np.asarray(mask).reshape(S, S)
    blk = m2[0:128, 0:128]                       # [q, k]
    tri = np.where(blk.T != 0, 0.0, MASK_NEG)    # [k, q]
    # additive mask is applied pre-scale, so divide by SCALE
    tri = (tri / SCALE).astype(BF16)

    Wq = np.asarray(Wq, np.float32)
    Wk = np.asarray(Wk, np.float32)
    Wv = np.asarray(Wv, np.float32)
    Wo = np.asarray(Wo, np.float32)
    bq = np.asarray(bq, np.float32)
    bk = np.asarray(bk, np.float32)

    in_maps = []
    for c in range(NCORES):
        b, g = divmod(c, 4)
        c0 = C * g
        bqk = np.stack([bq[c0:c0 + 128], bq[c0 + 128:c0 + 256],
                        bk[c0:c0 + 128], bk[c0 + 128:c0 + 256]], axis=1)
        in_maps.append({
            "xq_t": xt[("q", b)],
            "xk_t": xt[("k", b)],
            "xv_t": xt[("v", b)],
            "wq": Wq[:, c0:c0 + C].astype(BF16),
            "wk": Wk[:, c0:c0 + C].astype(BF16),
            "wv": Wv[:, c0:c0 + C].astype(BF16),
            "wo": np.ascontiguousarray(Wo[c0:c0 + C, :]).astype(BF16),
            "bqk": np.ascontiguousarray(bqk, dtype=np.float32),
            "tri": tri,
        })
    return in_maps


def gather_output(results, Wo, bv, bo):
    out = np.zeros((B, S, E), np.float32)
    for c in range(NCORES):
        b = c // 4
        out[b] += np.asarray(results[c]["out"], np.float32)
    # V-bias contributes + bv @ Wo to every row (softmax weights sum to 1)
    out += (np.asarray(bo, np.float32)
            + np.asarray(bv, np.float32) @ np.asarray(Wo, np.float32))
    return out


def kernel(**inputs) -> np.ndarray:
    from concourse.bass_utils import run_bass_kernel_spmd

    nc = get_program(1)
    in_maps = make_in_maps(**inputs)
    res = run_bass_kernel_spmd(nc, in_maps, core_ids=list(range(NCORES)))
    return gather_output(res.results, inputs["Wo"], inputs["bv"],
                         inputs["bo"])

